# revision 32
# baseline (speedup 1.0000x reference)
"""Trainium2 Bass kernel for GroupedQueryAttention (v3, bf16, early-start).

Sharding: 8 cores; core c owns KV head g=c and Q heads 4c..4c+3, both batch
elements. Each core computes its [2, 2048, 256] output slice; host concats.

Design:
  * hs is pre-transposed AND pre-cast to bf16 on the host (hsr layout:
    [b, s-chunk, d-in-tile(128), d-tile(16) x s(512)]), removing all 512
    PE transposes and their DVE drains from the device program.
  * All PE operands are bf16 (1.0 cycles/row at any ap size), PSUM fp32.
  * PV uses expS^T tiles as the STATIONARY and natural [V|1] as the moving
    ([128 k, 65]): output is ctx in natural [q, d] orientation with the
    softmax denominator in column 64 -> no output transposes, and PV costs
    65 rows per (k-tile, q-tile) instead of 512 per (k-tile, 512q).
  * exp runs on ACT in [128, 1024] chunks (double-buffered PSUM); ACT is
    the global bottleneck (~267us busy), so the schedule keeps it saturated:
    - early start: attention chunk (qc0,h0) k-tiles 0..7 begin right after
      hs-chunks 0,1 are projected; hs-chunks 2,3 interleave into those
      iterations, so the first exp fires ~35us earlier than a sequential
      phase-A/phase-C split;
    - batch 1's entire projection phase is interleaved into batch 0's
      attention loop (~2 PE instructions per k-tile iteration);
    - PV for chunk (qc,h) is software-pipelined into the scores/exp loop of
      the next chunk so ctx accumulation groups stay sequential per PSUM
      zero region (hardware allows one open group per 2KB bank).
"""

import sys
from contextlib import ExitStack

import numpy as np

sys.path.insert(0, "/opt/trn_rl_repo")

import concourse.bass as bass  # noqa: E402
import concourse.bacc as bacc  # noqa: E402
import concourse.tile as tile  # noqa: E402
from concourse import mybir  # noqa: E402
from concourse.bass_utils import run_bass_kernel_spmd  # noqa: E402

B = 2
S = 2048
D = 2048
HD = 64
NCORES = 8
QH = 4           # q heads per core
MCOLS = QH * HD  # 256 output cols per core

BF = mybir.dt.bfloat16
F32 = mybir.dt.float32
Exp = mybir.ActivationFunctionType.Exp

NDT = 16         # d tiles of 128
NSC = 4          # s chunks of 512 per batch
NKT = 16         # s_k tiles of 128
NQC = 2          # q chunks of 1024 per batch
QTPC = 8         # q tiles of 128 per q chunk


def build_nc():
    nc = bacc.Bacc("TRN2", target_bir_lowering=False, debug=False)

    hsr_d = nc.dram_tensor("hsr", [B, NSC, 128, NDT * 512], BF,
                           kind="ExternalInput")
    wqr_d = nc.dram_tensor("wqr", [128, NDT * 256], BF, kind="ExternalInput")
    wkvr_d = nc.dram_tensor("wkvr", [128, NDT * 128], BF, kind="ExternalInput")
    bq_d = nc.dram_tensor("bq", [128, 2], F32, kind="ExternalInput")
    bkv_d = nc.dram_tensor("bkv", [128, 1], F32, kind="ExternalInput")
    id_d = nc.dram_tensor("ident", [128, 128], BF, kind="ExternalInput")
    out_d = nc.dram_tensor("out", [B, S, MCOLS], F32, kind="ExternalOutput")

    with tile.TileContext(nc) as tc, ExitStack() as ctx:
        const = ctx.enter_context(tc.tile_pool(name="const", bufs=1))
        wqp = ctx.enter_context(tc.tile_pool(name="wqp", bufs=1))
        hstp = ctx.enter_context(tc.tile_pool(name="hstp", bufs=4))
        qtp = ctx.enter_context(tc.tile_pool(name="qtp", bufs=4))
        kvp = ctx.enter_context(tc.tile_pool(name="kvp", bufs=2))
        kthp = ctx.enter_context(tc.tile_pool(name="kthp", bufs=2))
        v1p = ctx.enter_context(tc.tile_pool(name="v1p", bufs=2 * NKT))
        expp = ctx.enter_context(tc.tile_pool(name="expp", bufs=38))
        recp = ctx.enter_context(tc.tile_pool(name="recp", bufs=4))
        outp = ctx.enter_context(tc.tile_pool(name="outp", bufs=16))
        psap = ctx.enter_context(tc.tile_pool(name="psap", bufs=2, space="PSUM"))
        pssp = ctx.enter_context(tc.tile_pool(name="pssp", bufs=2, space="PSUM"))
        ctxp = ctx.enter_context(tc.tile_pool(name="ctxp", bufs=1, space="PSUM"))

        # DMA order is the cold-start critical path: Wq first, then hs chunk
        # 0, so the first projection chain can begin ~9us in; everything else
        # follows.
        wq_sb = wqp.tile([128, NDT * 256], BF, tag="wq")
        nc.sync.dma_start(out=wq_sb[:], in_=wqr_d[:])
        hst = [[None] * NSC for _ in range(B)]
        for b in range(B):
            for sc in range(NSC):
                hst[b][sc] = hstp.tile([128, NDT * 512], BF, tag="hst",
                                       name=f"hst{b}_{sc}")
        nc.sync.dma_start(out=hst[0][0][:], in_=hsr_d[0, 0])
        wkv_sb = wqp.tile([128, NDT * 128], BF, tag="wkv")
        nc.sync.dma_start(out=wkv_sb[:], in_=wkvr_d[:])
        ident = const.tile([128, 128], BF, tag="ident")
        nc.sync.dma_start(out=ident[:], in_=id_d[:])
        bq_sb = const.tile([128, 2], F32, tag="bq")
        nc.sync.dma_start(out=bq_sb[:], in_=bq_d[:])
        bkv_sb = const.tile([128, 1], F32, tag="bkv")
        nc.sync.dma_start(out=bkv_sb[:], in_=bkv_d[:])
        zb = const.tile([128, 1], F32, tag="zb")
        nc.vector.memset(zb[:], 0.0)

        # PE p-state warmup: the tensor engine only reaches full clock after
        # ~3us of continuous work, and the first real chain otherwise pays
        # the slow ramp right when ACT is starved for scores.  Burn the
        # initial weight/hs DMA wait (~11us) on dummy matmuls over a memset
        # scratch tile so the ramp completes before real work arrives.
        scr = const.tile([128, 512], BF, tag="scr")
        nc.vector.memset(scr[:], 0.0)
        for _ in range(40):
            ps = psap.tile([128, 512], F32, tag="ps")
            nc.tensor.matmul(ps[:], scr[:, 0:128], scr[:], start=True,
                             stop=True)
        for b in range(B):
            for sc in range(NSC):
                if (b, sc) == (0, 0):
                    continue
                nc.sync.dma_start(out=hst[b][sc][:], in_=hsr_d[b, sc])

        qT = [[None, None] for _ in range(B)]
        kvT = [None] * B
        kth = [None] * B
        # V tiles pre-created with their ones column set while the DMAs of
        # the first hs chunks are still in flight (DVE is idle then).
        v1 = [[None] * NKT for _ in range(B)]
        for b in range(B):
            for kt in range(NKT):
                v = v1p.tile([128, 65], BF, tag="v1", name=f"v1_{b}_{kt}")
                nc.vector.memset(v[:, 64:65], 1.0)
                v1[b][kt] = v

        def init_b(b):
            qT[b][0] = qtp.tile([128, S], BF, tag="qt", name=f"qT{b}_0")
            qT[b][1] = qtp.tile([128, S], BF, tag="qt", name=f"qT{b}_1")
            kvT[b] = kvp.tile([128, S], BF, tag="kv", name=f"kvT{b}")
            kth[b] = kthp.tile([128, S], BF, tag="kth", name=f"kth{b}")

        def q_chain(b, sc, qc):
            hs_t = hst[b][sc]
            c0 = sc * 512
            ps = psap.tile([128, 512], F32, tag="ps")
            for dt_ in range(NDT):
                nc.tensor.matmul(
                    ps[:],
                    wq_sb[:, dt_ * 256 + qc * 128:dt_ * 256 + (qc + 1) * 128],
                    hs_t[:, dt_ * 512:(dt_ + 1) * 512],
                    start=(dt_ == 0), stop=(dt_ == NDT - 1),
                )
                yield
            nc.vector.tensor_scalar_add(
                qT[b][qc][:, c0:c0 + 512], ps[:], bq_sb[:, qc:qc + 1])
            yield

        def chunk_q0kv(b, sc):
            """Q heads 0/1 + KV projections, kth copy and V tiles for hs
            chunk sc.  Everything attention on heads 0/1 needs; the heads
            2/3 projection (q_chain qc=1) can run much later."""
            hs_t = hst[b][sc]
            c0 = sc * 512
            yield from q_chain(b, sc, 0)
            ps = psap.tile([128, 512], F32, tag="ps")
            for dt_ in range(NDT):
                nc.tensor.matmul(
                    ps[:], wkv_sb[:, dt_ * 128:(dt_ + 1) * 128],
                    hs_t[:, dt_ * 512:(dt_ + 1) * 512],
                    start=(dt_ == 0), stop=(dt_ == NDT - 1),
                )
                yield
            nc.vector.tensor_scalar_add(
                kvT[b][:, c0:c0 + 512], ps[:], bkv_sb[:])
            yield
            # K^T rows shifted to partitions 64:127 for odd heads; issued
            # from the Pool queue so the SP queue (hsT loads) is not blocked.
            nc.gpsimd.dma_start(out=kth[b][64:128, c0:c0 + 512],
                                in_=kvT[b][0:64, c0:c0 + 512])
            yield
            # V natural tiles [s_k 128, 64]; the ones column was pre-set.
            for kt in range(sc * 4, sc * 4 + 4):
                pst = psap.tile([128, 512], BF, tag="ps", name=f"pst{b}_{kt}")
                nc.tensor.transpose(
                    pst[:, 0:64], kvT[b][64:128, kt * 128:(kt + 1) * 128],
                    ident[64:128, 64:128],
                )
                yield
                nc.vector.tensor_copy(v1[b][kt][:, 0:64], pst[:, 0:64])
                yield

        def gen_a(b):
            init_b(b)
            for sc in range(NSC):
                yield from chunk_q0kv(b, sc)
                yield from q_chain(b, sc, 1)

        # ---- phase C machinery ----
        outt = {0: [None] * (NQC * QTPC), 1: [None] * (NQC * QTPC)}
        exs = {}
        ctxs = {}

        def coff(qi):
            # qi 0..6 packed in bank 0; qi 7 at the bank-1 boundary so no
            # accumulation group straddles a PSUM bank.
            return qi * 65 if qi < 7 else 512

        def ex_stat(bkey, kt, qi):
            for ex, qi0, nqi in exs[bkey][kt]:
                if qi0 <= qi < qi0 + nqi:
                    j = qi - qi0
                    return ex[:, j * 128:(j + 1) * 128]
            raise AssertionError(f"no exp span for {bkey} kt={kt} qi={qi}")

        def pv_chunk(bkey, it, swap=False):
            # it 0..15: two passes of 8 k-tiles per q-tile qi = it//2.
            # swap=True consumes k-tiles 8..15 on the first pass (used when
            # the producing chunk emitted its exps in swapped order).
            b, qc, h = bkey
            ctx_t = ctxs[bkey]
            qi = it // 2
            base = (it % 2) * 8
            if swap:
                base = 8 - base
            for k2 in range(8):
                kt = base + k2
                nc.tensor.matmul(
                    ctx_t[:, coff(qi):coff(qi) + 65],
                    ex_stat(bkey, kt, qi),
                    v1[b][kt][:],
                    start=(it % 2 == 0 and k2 == 0),
                    stop=(it % 2 == 1 and k2 == 7),
                )

        def normalize_qi(bkey, qi):
            b, qc, h = bkey
            ctx_t = ctxs[bkey]
            qtile = qc * QTPC + qi
            if h == 0:
                outt[b][qtile] = outp.tile([128, MCOLS], F32, tag="out",
                                           name=f"outt{b}_{qtile}")
            rec = recp.tile([128, 1], F32, tag="rec")
            nc.vector.reciprocal(
                rec[:], ctx_t[:, coff(qi) + 64:coff(qi) + 65])
            nc.vector.tensor_scalar_mul(
                outt[b][qtile][:, h * 64:(h + 1) * 64],
                ctx_t[:, coff(qi):coff(qi) + 64], rec[:])
            if h == QH - 1:
                nc.sync.dma_start(
                    out=out_d[b, qtile * 128:(qtile + 1) * 128, :],
                    in_=outt[b][qtile][:])

        def begin_chunk(bkey):
            b, qc, h = bkey
            ctxs[bkey] = ctxp.tile([128, 577], F32, tag="ctx",
                                   name=f"ctx{b}_{qc}_{h}")
            exs[bkey] = []

        def score_exp(bkey, kt, qi0=0, nqi=QTPC):
            b, qc, h = bkey
            r0 = (h % 2) * 64
            kmat = kvT[b] if r0 == 0 else kth[b]
            qt = qT[b][h // 2]
            w = nqi * 128
            pss = pssp.tile([128, 1024], F32, tag="pss")
            for j in range(0, w, 512):
                jw = min(512, w - j)
                q0 = qc * 1024 + qi0 * 128 + j
                nc.tensor.matmul(
                    pss[:, j:j + jw],
                    kmat[r0:r0 + 64, kt * 128:(kt + 1) * 128],
                    qt[r0:r0 + 64, q0:q0 + jw],
                    start=True, stop=True,
                )
            ex = expp.tile([128, 1024], BF, tag="ex")
            nc.scalar.activation(ex[:, 0:w], pss[:, 0:w], Exp, bias=zb[:])
            while len(exs[bkey]) <= kt:
                exs[bkey].append([])
            exs[bkey][kt].append((ex, qi0, nqi))

        def finish_chunk(bkey):
            del exs[bkey]
            del ctxs[bkey]

        def advance(gen, n):
            if gen is None:
                return None
            for _ in range(n):
                if next(gen, StopIteration) is StopIteration:
                    return None
            return gen

        # ---- emission ----
        # Batch 0 prologue: after hs chunk 0's Q0/KV projections, scores/exp
        # for (qc0, h0) start immediately on the q/k tiles that exist,
        # widening as later chunks land.  Each 4-iteration phase interleaves
        # exactly the projection work the NEXT phase needs, so batch 0's
        # bulk is spread across the whole prologue instead of serializing
        # in front of it.
        init_b(0)
        for _ in chunk_q0kv(0, 0):
            pass
        key00 = (0, 0, 0)
        key01 = (0, 0, 1)
        begin_chunk(key00)
        begin_chunk(key01)
        phases = [
            (key00, range(4), 0, 4, chunk_q0kv(0, 1), 11),  # k 0:512 q 0:512
            (key00, range(4), 4, 4, chunk_q0kv(0, 2), 11),  # k 0:512 q 512:1024
            (key00, range(4, 8), 0, 8, chunk_q0kv(0, 3), 11),
            (key00, range(8, 12), 0, 8, q_chain(0, 0, 1), 5),
            (key00, range(12, 16), 0, 8, q_chain(0, 1, 1), 5),
            # head 1 shares q-chunk 0 and only needs kth (K^T copy), which
            # landed with each hs chunk: its first 8 k-tiles run here too,
            # doubling the exp work available while projections drain.
            (key01, range(4), 0, 8, q_chain(0, 2, 1), 5),
            (key01, range(4, 8), 0, 8, q_chain(0, 3, 1), 5),
        ]
        for key, kts, qi0, nqi, g, rate in phases:
            for kt in kts:
                score_exp(key, kt, qi0, nqi)
                g = advance(g, rate)
            if g is not None:
                for _ in g:
                    pass

        # Steady state: remaining 7 chunks of batch 0 with the deferred
        # batch-0 head-2/3 projections and batch 1's full projection phase
        # interleaved at an adaptive rate, then batch 1's chunks.
        order0 = [(0, qc, h) for qc in range(NQC) for h in range(QH)][1:]
        order1 = [(1, qc, h) for qc in range(NQC) for h in range(QH)]

        def steady_units():
            init_b(1)
            for sc in range(NSC):
                yield from chunk_q0kv(1, sc)
                if sc < 2:
                    # heads 2/3 projections for hs chunks 2,3 are deferred
                    # into batch 1's own attention loop (needed at its
                    # iteration 96, not before it starts).
                    yield from q_chain(1, sc, 1)

        g1 = steady_units()
        prev = key00
        for bkey in order0:
            if bkey != key01:
                begin_chunk(bkey)
            for kt in range(NKT):
                # key01's k-tiles 0..7 were already emitted in the prologue;
                # its remaining 8..15 run during the first half of its loop
                # so ACT keeps receiving one exp per iteration.
                if bkey != key01:
                    score_exp(bkey, kt)
                elif kt < 8:
                    score_exp(key01, kt + 8)
                pv_chunk(prev, kt)
                if kt % 2 == 1:
                    normalize_qi(prev, kt // 2)
                g1 = advance(g1, 2)
            finish_chunk(prev)
            prev = bkey
        # flush batch 1 projections before its attention begins
        if g1 is not None:
            for _ in g1:
                pass
        def b1_tail():
            yield from q_chain(1, 2, 1)
            yield from q_chain(1, 3, 1)

        gt = b1_tail()
        for bkey in order1:
            last = bkey == order1[-1]
            begin_chunk(bkey)
            for i, kt in enumerate(range(NKT)):
                # The final chunk emits k-tiles 8..15 first so its PV flush
                # (which consumes the late k-tiles on odd passes) is never
                # waiting on the exp backlog at the very end.
                score_exp(bkey, (kt + 8) % NKT if last else kt)
                pv_chunk(prev, kt)
                if kt % 2 == 1:
                    normalize_qi(prev, kt // 2)
                gt = advance(gt, 2)
            finish_chunk(prev)
            prev = bkey
        for it in range(NKT):
            pv_chunk(prev, it, swap=True)
            if it % 2 == 1:
                normalize_qi(prev, it // 2)
        finish_chunk(prev)

    nc.compile()
    return nc


def make_in_maps(hidden_states, Wq, bq, Wk, bk, Wv, bv):
    bf = mybir.dt.np(BF)
    hs = np.asarray(hidden_states, dtype=np.float32)
    Wq = np.asarray(Wq, dtype=np.float32)
    bq = np.asarray(bq, dtype=np.float32)
    Wk = np.asarray(Wk, dtype=np.float32)
    bk = np.asarray(bk, dtype=np.float32)
    Wv = np.asarray(Wv, dtype=np.float32)
    bv = np.asarray(bv, dtype=np.float32)
    sc = 1.0 / np.sqrt(np.float32(HD))
    # [b, sc, p(d in tile), t(d tile), j(s in chunk)] -> [2, 4, 128, 8192]
    hsr = np.ascontiguousarray(
        hs.reshape(B, NSC, 512, NDT, 128).transpose(0, 1, 4, 3, 2)
        .reshape(B, NSC, 128, NDT * 512).astype(bf))
    ident = np.eye(128, dtype=bf)
    in_maps = []
    for c in range(NCORES):
        qs = slice(c * MCOLS, (c + 1) * MCOLS)
        ks = slice(c * HD, (c + 1) * HD)
        wqs = (Wq[:, qs] * sc).astype(bf)
        wqr = np.ascontiguousarray(
            wqs.reshape(NDT, 128, MCOLS).transpose(1, 0, 2)
            .reshape(128, NDT * MCOLS))
        wkvs = np.concatenate([Wk[:, ks], Wv[:, ks]], axis=1).astype(bf)
        wkvr = np.ascontiguousarray(
            wkvs.reshape(NDT, 128, 128).transpose(1, 0, 2)
            .reshape(128, NDT * 128))
        bq_c = np.ascontiguousarray((bq[qs] * sc).reshape(2, 128).T)
        in_maps.append({
            "hsr": hsr,
            "wqr": wqr,
            "wkvr": wkvr,
            "bq": bq_c,
            "bkv": np.concatenate([bk[ks], bv[ks]]).reshape(128, 1),
            "ident": ident,
        })
    return in_maps


_NC_CACHE = {}


def get_nc():
    if "nc" not in _NC_CACHE:
        _NC_CACHE["nc"] = build_nc()
    return _NC_CACHE["nc"]


def kernel(hidden_states, Wq, bq, Wk, bk, Wv, bv):
    nc = get_nc()
    in_maps = make_in_maps(hidden_states, Wq, bq, Wk, bk, Wv, bv)
    res = run_bass_kernel_spmd(nc, in_maps, list(range(NCORES)))
    outs = [np.asarray(r["out"], dtype=np.float32) for r in res.results]
    return np.concatenate(outs, axis=-1)


# revision 35
# speedup vs baseline: 1.0201x; 1.0201x over previous
"""Trainium2 Bass kernel for GroupedQueryAttention (v3, bf16, early-start).

Sharding: 8 cores; core c owns KV head g=c and Q heads 4c..4c+3, both batch
elements. Each core computes its [2, 2048, 256] output slice; host concats.

Design:
  * hs is pre-transposed AND pre-cast to bf16 on the host (hsr layout:
    [b, s-chunk, d-in-tile(128), d-tile(16) x s(512)]), removing all 512
    PE transposes and their DVE drains from the device program.
  * All PE operands are bf16 (1.0 cycles/row at any ap size), PSUM fp32.
  * PV uses expS^T tiles as the STATIONARY and natural [V|1] as the moving
    ([128 k, 65]): output is ctx in natural [q, d] orientation with the
    softmax denominator in column 64 -> no output transposes, and PV costs
    65 rows per (k-tile, q-tile) instead of 512 per (k-tile, 512q).
  * exp runs on ACT in [128, 1024] chunks (double-buffered PSUM); ACT is
    the global bottleneck (~267us busy), so the schedule keeps it saturated:
    - early start: attention chunk (qc0,h0) k-tiles 0..7 begin right after
      hs-chunks 0,1 are projected; hs-chunks 2,3 interleave into those
      iterations, so the first exp fires ~35us earlier than a sequential
      phase-A/phase-C split;
    - batch 1's entire projection phase is interleaved into batch 0's
      attention loop (~2 PE instructions per k-tile iteration);
    - PV for chunk (qc,h) is software-pipelined into the scores/exp loop of
      the next chunk so ctx accumulation groups stay sequential per PSUM
      zero region (hardware allows one open group per 2KB bank).
"""

import sys
from contextlib import ExitStack

import numpy as np

sys.path.insert(0, "/opt/trn_rl_repo")

import concourse.bass as bass  # noqa: E402
import concourse.bacc as bacc  # noqa: E402
import concourse.tile as tile  # noqa: E402
from concourse import mybir  # noqa: E402
from concourse.bass_utils import run_bass_kernel_spmd  # noqa: E402

B = 2
S = 2048
D = 2048
HD = 64
NCORES = 8
QH = 4           # q heads per core
MCOLS = QH * HD  # 256 output cols per core

BF = mybir.dt.bfloat16
F32 = mybir.dt.float32
Exp = mybir.ActivationFunctionType.Exp

NDT = 16         # d tiles of 128
NSC = 4          # s chunks of 512 per batch
NKT = 16         # s_k tiles of 128
NQC = 2          # q chunks of 1024 per batch
QTPC = 8         # q tiles of 128 per q chunk


def build_nc():
    nc = bacc.Bacc("TRN2", target_bir_lowering=False, debug=False)

    hsr_d = nc.dram_tensor("hsr", [B, NSC, 128, NDT * 512], BF,
                           kind="ExternalInput")
    wqr_d = nc.dram_tensor("wqr", [128, NDT * 256], BF, kind="ExternalInput")
    wkvr_d = nc.dram_tensor("wkvr", [128, NDT * 128], BF, kind="ExternalInput")
    bq_d = nc.dram_tensor("bq", [128, 2], F32, kind="ExternalInput")
    bkv_d = nc.dram_tensor("bkv", [128, 1], F32, kind="ExternalInput")
    id_d = nc.dram_tensor("ident", [128, 128], BF, kind="ExternalInput")
    out_d = nc.dram_tensor("out", [B, S, MCOLS], F32, kind="ExternalOutput")

    with tile.TileContext(nc) as tc, ExitStack() as ctx:
        const = ctx.enter_context(tc.tile_pool(name="const", bufs=1))
        wqp = ctx.enter_context(tc.tile_pool(name="wqp", bufs=1))
        hstp = ctx.enter_context(tc.tile_pool(name="hstp", bufs=4))
        qtp = ctx.enter_context(tc.tile_pool(name="qtp", bufs=4))
        kvp = ctx.enter_context(tc.tile_pool(name="kvp", bufs=2))
        kthp = ctx.enter_context(tc.tile_pool(name="kthp", bufs=2))
        v1p = ctx.enter_context(tc.tile_pool(name="v1p", bufs=2 * NKT))
        expp = ctx.enter_context(tc.tile_pool(name="expp", bufs=38))
        recp = ctx.enter_context(tc.tile_pool(name="recp", bufs=4))
        outp = ctx.enter_context(tc.tile_pool(name="outp", bufs=16))
        psap = ctx.enter_context(tc.tile_pool(name="psap", bufs=2, space="PSUM"))
        pssp = ctx.enter_context(tc.tile_pool(name="pssp", bufs=2, space="PSUM"))
        ctxp = ctx.enter_context(tc.tile_pool(name="ctxp", bufs=1, space="PSUM"))

        # DMA order is the cold-start critical path: Wq first, then hs chunk
        # 0, so the first projection chain can begin ~9us in; everything else
        # follows.
        wq_sb = wqp.tile([128, NDT * 256], BF, tag="wq")
        nc.sync.dma_start(out=wq_sb[:], in_=wqr_d[:])
        hst = [[None] * NSC for _ in range(B)]
        for b in range(B):
            for sc in range(NSC):
                hst[b][sc] = hstp.tile([128, NDT * 512], BF, tag="hst",
                                       name=f"hst{b}_{sc}")
        nc.sync.dma_start(out=hst[0][0][:], in_=hsr_d[0, 0])
        wkv_sb = wqp.tile([128, NDT * 128], BF, tag="wkv")
        nc.sync.dma_start(out=wkv_sb[:], in_=wkvr_d[:])
        ident = const.tile([128, 128], BF, tag="ident")
        nc.sync.dma_start(out=ident[:], in_=id_d[:])
        bq_sb = const.tile([128, 2], F32, tag="bq")
        nc.sync.dma_start(out=bq_sb[:], in_=bq_d[:])
        bkv_sb = const.tile([128, 1], F32, tag="bkv")
        nc.sync.dma_start(out=bkv_sb[:], in_=bkv_d[:])
        zb = const.tile([128, 1], F32, tag="zb")
        nc.vector.memset(zb[:], 0.0)

        # PE p-state warmup: the tensor engine only reaches full clock after
        # ~3us of continuous work, and the first real chain otherwise pays
        # the slow ramp right when ACT is starved for scores.  Burn the
        # initial weight/hs DMA wait (~11us) on dummy matmuls over a memset
        # scratch tile so the ramp completes before real work arrives.
        scr = const.tile([128, 512], BF, tag="scr")
        nc.vector.memset(scr[:], 0.0)
        for _ in range(40):
            ps = psap.tile([128, 512], F32, tag="ps")
            nc.tensor.matmul(ps[:], scr[:, 0:128], scr[:], start=True,
                             stop=True)
        for b in range(B):
            for sc in range(NSC):
                if (b, sc) == (0, 0):
                    continue
                nc.sync.dma_start(out=hst[b][sc][:], in_=hsr_d[b, sc])

        qT = [[None, None] for _ in range(B)]
        kvT = [None] * B
        kth = [None] * B
        # V tiles pre-created with their ones column set while the DMAs of
        # the first hs chunks are still in flight (DVE is idle then).
        v1 = [[None] * NKT for _ in range(B)]
        for b in range(B):
            for kt in range(NKT):
                v = v1p.tile([128, 65], BF, tag="v1", name=f"v1_{b}_{kt}")
                nc.vector.memset(v[:, 64:65], 1.0)
                v1[b][kt] = v

        def init_b(b):
            qT[b][0] = qtp.tile([128, S], BF, tag="qt", name=f"qT{b}_0")
            qT[b][1] = qtp.tile([128, S], BF, tag="qt", name=f"qT{b}_1")
            kvT[b] = kvp.tile([128, S], BF, tag="kv", name=f"kvT{b}")
            kth[b] = kthp.tile([128, S], BF, tag="kth", name=f"kth{b}")

        def q_chain(b, sc, qc):
            hs_t = hst[b][sc]
            c0 = sc * 512
            ps = psap.tile([128, 512], F32, tag="ps")
            for dt_ in range(NDT):
                nc.tensor.matmul(
                    ps[:],
                    wq_sb[:, dt_ * 256 + qc * 128:dt_ * 256 + (qc + 1) * 128],
                    hs_t[:, dt_ * 512:(dt_ + 1) * 512],
                    start=(dt_ == 0), stop=(dt_ == NDT - 1),
                )
                yield
            nc.vector.tensor_scalar_add(
                qT[b][qc][:, c0:c0 + 512], ps[:], bq_sb[:, qc:qc + 1])
            yield

        def chunk_q0kv(b, sc):
            """Q heads 0/1 + KV projections, kth copy and V tiles for hs
            chunk sc.  Everything attention on heads 0/1 needs; the heads
            2/3 projection (q_chain qc=1) can run much later."""
            hs_t = hst[b][sc]
            c0 = sc * 512
            yield from q_chain(b, sc, 0)
            ps = psap.tile([128, 512], F32, tag="ps")
            for dt_ in range(NDT):
                nc.tensor.matmul(
                    ps[:], wkv_sb[:, dt_ * 128:(dt_ + 1) * 128],
                    hs_t[:, dt_ * 512:(dt_ + 1) * 512],
                    start=(dt_ == 0), stop=(dt_ == NDT - 1),
                )
                yield
            nc.vector.tensor_scalar_add(
                kvT[b][:, c0:c0 + 512], ps[:], bkv_sb[:])
            yield
            # K^T rows shifted to partitions 64:127 for odd heads; issued
            # from the Pool queue so the SP queue (hsT loads) is not blocked.
            nc.gpsimd.dma_start(out=kth[b][64:128, c0:c0 + 512],
                                in_=kvT[b][0:64, c0:c0 + 512])
            yield
            # V natural tiles [s_k 128, 64]; the ones column was pre-set.
            for kt in range(sc * 4, sc * 4 + 4):
                pst = psap.tile([128, 512], BF, tag="ps", name=f"pst{b}_{kt}")
                nc.tensor.transpose(
                    pst[:, 0:64], kvT[b][64:128, kt * 128:(kt + 1) * 128],
                    ident[64:128, 64:128],
                )
                yield
                nc.vector.tensor_copy(v1[b][kt][:, 0:64], pst[:, 0:64])
                yield

        def gen_a(b):
            init_b(b)
            for sc in range(NSC):
                yield from chunk_q0kv(b, sc)
                yield from q_chain(b, sc, 1)

        # ---- phase C machinery ----
        outt = {0: [None] * (NQC * QTPC), 1: [None] * (NQC * QTPC)}
        exs = {}
        ctxs = {}

        def coff(qi):
            # qi 0..6 packed in bank 0; qi 7 at the bank-1 boundary so no
            # accumulation group straddles a PSUM bank.
            return qi * 65 if qi < 7 else 512

        def ex_stat(bkey, kt, qi):
            for ex, qi0, nqi in exs[bkey][kt]:
                if qi0 <= qi < qi0 + nqi:
                    j = qi - qi0
                    return ex[:, j * 128:(j + 1) * 128]
            raise AssertionError(f"no exp span for {bkey} kt={kt} qi={qi}")

        def pv_chunk(bkey, it, swap=False):
            # it 0..15: two passes of 8 k-tiles per q-tile qi = it//2.
            # swap=True consumes k-tiles 8..15 on the first pass (used when
            # the producing chunk emitted its exps in swapped order).
            b, qc, h = bkey
            ctx_t = ctxs[bkey]
            qi = it // 2
            base = (it % 2) * 8
            if swap:
                base = 8 - base
            for k2 in range(8):
                kt = base + k2
                nc.tensor.matmul(
                    ctx_t[:, coff(qi):coff(qi) + 65],
                    ex_stat(bkey, kt, qi),
                    v1[b][kt][:],
                    start=(it % 2 == 0 and k2 == 0),
                    stop=(it % 2 == 1 and k2 == 7),
                )

        def normalize_qi(bkey, qi):
            b, qc, h = bkey
            ctx_t = ctxs[bkey]
            qtile = qc * QTPC + qi
            if h == 0:
                outt[b][qtile] = outp.tile([128, MCOLS], F32, tag="out",
                                           name=f"outt{b}_{qtile}")
            rec = recp.tile([128, 1], F32, tag="rec")
            nc.vector.reciprocal(
                rec[:], ctx_t[:, coff(qi) + 64:coff(qi) + 65])
            nc.vector.tensor_scalar_mul(
                outt[b][qtile][:, h * 64:(h + 1) * 64],
                ctx_t[:, coff(qi):coff(qi) + 64], rec[:])
            if h == QH - 1:
                nc.sync.dma_start(
                    out=out_d[b, qtile * 128:(qtile + 1) * 128, :],
                    in_=outt[b][qtile][:])

        def begin_chunk(bkey):
            b, qc, h = bkey
            ctxs[bkey] = ctxp.tile([128, 577], F32, tag="ctx",
                                   name=f"ctx{b}_{qc}_{h}")
            exs[bkey] = []

        def score_exp(bkey, kt, qi0=0, nqi=QTPC):
            b, qc, h = bkey
            r0 = (h % 2) * 64
            kmat = kvT[b] if r0 == 0 else kth[b]
            qt = qT[b][h // 2]
            w = nqi * 128
            pss = pssp.tile([128, 1024], F32, tag="pss")
            for j in range(0, w, 512):
                jw = min(512, w - j)
                q0 = qc * 1024 + qi0 * 128 + j
                nc.tensor.matmul(
                    pss[:, j:j + jw],
                    kmat[r0:r0 + 64, kt * 128:(kt + 1) * 128],
                    qt[r0:r0 + 64, q0:q0 + jw],
                    start=True, stop=True,
                )
            ex = expp.tile([128, 1024], BF, tag="ex")
            nc.scalar.activation(ex[:, 0:w], pss[:, 0:w], Exp, bias=zb[:])
            while len(exs[bkey]) <= kt:
                exs[bkey].append([])
            exs[bkey][kt].append((ex, qi0, nqi))

        def finish_chunk(bkey):
            del exs[bkey]
            del ctxs[bkey]

        def advance(gen, n):
            if gen is None:
                return None
            for _ in range(n):
                if next(gen, StopIteration) is StopIteration:
                    return None
            return gen

        # ---- emission ----
        # Batch 0 prologue: after hs chunk 0's Q0/KV projections, scores/exp
        # for (qc0, h0) start immediately on the q/k tiles that exist,
        # widening as later chunks land.  Each 4-iteration phase interleaves
        # exactly the projection work the NEXT phase needs, so batch 0's
        # bulk is spread across the whole prologue instead of serializing
        # in front of it.
        init_b(0)
        for _ in chunk_q0kv(0, 0):
            pass
        key00 = (0, 0, 0)
        begin_chunk(key00)
        phases = [
            (range(4), 0, 4, chunk_q0kv(0, 1), 11),   # k 0:512 x q 0:512
            (range(4), 4, 4, chunk_q0kv(0, 2), 11),   # k 0:512 x q 512:1024
            (range(4, 8), 0, 8, chunk_q0kv(0, 3), 11),
            (range(8, 12), 0, 8, q_chain(0, 0, 1), 5),
            (range(12, 16), 0, 8, q_chain(0, 1, 1), 5),
        ]
        for kts, qi0, nqi, g, rate in phases:
            for kt in kts:
                score_exp(key00, kt, qi0, nqi)
                g = advance(g, rate)
            if g is not None:
                for _ in g:
                    pass

        # Steady state: remaining 7 chunks of batch 0 with the deferred
        # batch-0 head-2/3 projections and batch 1's full projection phase
        # interleaved at an adaptive rate, then batch 1's chunks.
        order0 = [(0, qc, h) for qc in range(NQC) for h in range(QH)][1:]
        order1 = [(1, qc, h) for qc in range(NQC) for h in range(QH)]

        def steady_units():
            yield from q_chain(0, 2, 1)
            yield from q_chain(0, 3, 1)
            init_b(1)
            for sc in range(NSC):
                yield from chunk_q0kv(1, sc)
                if sc < 2:
                    # heads 2/3 projections for hs chunks 2,3 are deferred
                    # into batch 1's own attention loop (needed at its
                    # iteration 96, not before it starts).
                    yield from q_chain(1, sc, 1)

        g1 = steady_units()
        prev = key00
        for bkey in order0:
            begin_chunk(bkey)
            for kt in range(NKT):
                score_exp(bkey, kt)
                pv_chunk(prev, kt)
                if kt % 2 == 1:
                    normalize_qi(prev, kt // 2)
                g1 = advance(g1, 2)
            finish_chunk(prev)
            prev = bkey
        # flush batch 1 projections before its attention begins
        if g1 is not None:
            for _ in g1:
                pass
        def b1_tail():
            yield from q_chain(1, 2, 1)
            yield from q_chain(1, 3, 1)

        gt = b1_tail()
        for bkey in order1:
            last = bkey == order1[-1]
            begin_chunk(bkey)
            for i, kt in enumerate(range(NKT)):
                # The final chunk emits k-tiles 8..15 first so its PV flush
                # (which consumes the late k-tiles on odd passes) is never
                # waiting on the exp backlog at the very end.
                score_exp(bkey, (kt + 8) % NKT if last else kt)
                pv_chunk(prev, kt)
                if kt % 2 == 1:
                    normalize_qi(prev, kt // 2)
                gt = advance(gt, 2)
            finish_chunk(prev)
            prev = bkey
        for it in range(NKT):
            pv_chunk(prev, it, swap=True)
            if it % 2 == 1:
                normalize_qi(prev, it // 2)
        finish_chunk(prev)

    nc.compile()
    return nc


def make_in_maps(hidden_states, Wq, bq, Wk, bk, Wv, bv):
    bf = mybir.dt.np(BF)
    hs = np.asarray(hidden_states, dtype=np.float32)
    Wq = np.asarray(Wq, dtype=np.float32)
    bq = np.asarray(bq, dtype=np.float32)
    Wk = np.asarray(Wk, dtype=np.float32)
    bk = np.asarray(bk, dtype=np.float32)
    Wv = np.asarray(Wv, dtype=np.float32)
    bv = np.asarray(bv, dtype=np.float32)
    sc = 1.0 / np.sqrt(np.float32(HD))
    # [b, sc, p(d in tile), t(d tile), j(s in chunk)] -> [2, 4, 128, 8192]
    hsr = np.ascontiguousarray(
        hs.reshape(B, NSC, 512, NDT, 128).transpose(0, 1, 4, 3, 2)
        .reshape(B, NSC, 128, NDT * 512).astype(bf))
    ident = np.eye(128, dtype=bf)
    in_maps = []
    for c in range(NCORES):
        qs = slice(c * MCOLS, (c + 1) * MCOLS)
        ks = slice(c * HD, (c + 1) * HD)
        wqs = (Wq[:, qs] * sc).astype(bf)
        wqr = np.ascontiguousarray(
            wqs.reshape(NDT, 128, MCOLS).transpose(1, 0, 2)
            .reshape(128, NDT * MCOLS))
        wkvs = np.concatenate([Wk[:, ks], Wv[:, ks]], axis=1).astype(bf)
        wkvr = np.ascontiguousarray(
            wkvs.reshape(NDT, 128, 128).transpose(1, 0, 2)
            .reshape(128, NDT * 128))
        bq_c = np.ascontiguousarray((bq[qs] * sc).reshape(2, 128).T)
        in_maps.append({
            "hsr": hsr,
            "wqr": wqr,
            "wkvr": wkvr,
            "bq": bq_c,
            "bkv": np.concatenate([bk[ks], bv[ks]]).reshape(128, 1),
            "ident": ident,
        })
    return in_maps


_NC_CACHE = {}


def get_nc():
    if "nc" not in _NC_CACHE:
        _NC_CACHE["nc"] = build_nc()
    return _NC_CACHE["nc"]


def kernel(hidden_states, Wq, bq, Wk, bk, Wv, bv):
    nc = get_nc()
    in_maps = make_in_maps(hidden_states, Wq, bq, Wk, bk, Wv, bv)
    res = run_bass_kernel_spmd(nc, in_maps, list(range(NCORES)))
    outs = [np.asarray(r["out"], dtype=np.float32) for r in res.results]
    return np.concatenate(outs, axis=-1)


# revision 37
# speedup vs baseline: 1.0239x; 1.0038x over previous
"""Trainium2 Bass kernel for GroupedQueryAttention (v3, bf16, early-start).

Sharding: 8 cores; core c owns KV head g=c and Q heads 4c..4c+3, both batch
elements. Each core computes its [2, 2048, 256] output slice; host concats.

Design:
  * hs is pre-transposed AND pre-cast to bf16 on the host (hsr layout:
    [b, s-chunk, d-in-tile(128), d-tile(16) x s(512)]), removing all 512
    PE transposes and their DVE drains from the device program.
  * All PE operands are bf16 (1.0 cycles/row at any ap size), PSUM fp32.
  * PV uses expS^T tiles as the STATIONARY and natural [V|1] as the moving
    ([128 k, 65]): output is ctx in natural [q, d] orientation with the
    softmax denominator in column 64 -> no output transposes, and PV costs
    65 rows per (k-tile, q-tile) instead of 512 per (k-tile, 512q).
  * exp runs on ACT in [128, 1024] chunks (double-buffered PSUM); ACT is
    the global bottleneck (~267us busy), so the schedule keeps it saturated:
    - early start: attention chunk (qc0,h0) k-tiles 0..7 begin right after
      hs-chunks 0,1 are projected; hs-chunks 2,3 interleave into those
      iterations, so the first exp fires ~35us earlier than a sequential
      phase-A/phase-C split;
    - batch 1's entire projection phase is interleaved into batch 0's
      attention loop (~2 PE instructions per k-tile iteration);
    - PV for chunk (qc,h) is software-pipelined into the scores/exp loop of
      the next chunk so ctx accumulation groups stay sequential per PSUM
      zero region (hardware allows one open group per 2KB bank).
"""

import sys
from contextlib import ExitStack

import numpy as np

sys.path.insert(0, "/opt/trn_rl_repo")

import concourse.bass as bass  # noqa: E402
import concourse.bacc as bacc  # noqa: E402
import concourse.tile as tile  # noqa: E402
from concourse import mybir  # noqa: E402
from concourse.bass_utils import run_bass_kernel_spmd  # noqa: E402

B = 2
S = 2048
D = 2048
HD = 64
NCORES = 8
QH = 4           # q heads per core
MCOLS = QH * HD  # 256 output cols per core

BF = mybir.dt.bfloat16
F32 = mybir.dt.float32
Exp = mybir.ActivationFunctionType.Exp

NDT = 16         # d tiles of 128
NSC = 4          # s chunks of 512 per batch
NKT = 16         # s_k tiles of 128
NQC = 2          # q chunks of 1024 per batch
QTPC = 8         # q tiles of 128 per q chunk


def build_nc():
    nc = bacc.Bacc("TRN2", target_bir_lowering=False, debug=False)

    hsr_d = nc.dram_tensor("hsr", [B, NSC, 128, NDT * 512], BF,
                           kind="ExternalInput")
    wqr_d = nc.dram_tensor("wqr", [128, NDT * 256], BF, kind="ExternalInput")
    wkvr_d = nc.dram_tensor("wkvr", [128, NDT * 128], BF, kind="ExternalInput")
    bq_d = nc.dram_tensor("bq", [128, 2], F32, kind="ExternalInput")
    bkv_d = nc.dram_tensor("bkv", [128, 1], F32, kind="ExternalInput")
    id_d = nc.dram_tensor("ident", [128, 128], BF, kind="ExternalInput")
    out_d = nc.dram_tensor("out", [B, S, MCOLS], F32, kind="ExternalOutput")

    with tile.TileContext(nc) as tc, ExitStack() as ctx:
        const = ctx.enter_context(tc.tile_pool(name="const", bufs=1))
        wqp = ctx.enter_context(tc.tile_pool(name="wqp", bufs=1))
        hstp = ctx.enter_context(tc.tile_pool(name="hstp", bufs=4))
        qtp = ctx.enter_context(tc.tile_pool(name="qtp", bufs=4))
        kvp = ctx.enter_context(tc.tile_pool(name="kvp", bufs=2))
        kthp = ctx.enter_context(tc.tile_pool(name="kthp", bufs=2))
        v1p = ctx.enter_context(tc.tile_pool(name="v1p", bufs=2 * NKT))
        expp = ctx.enter_context(tc.tile_pool(name="expp", bufs=38))
        recp = ctx.enter_context(tc.tile_pool(name="recp", bufs=4))
        outp = ctx.enter_context(tc.tile_pool(name="outp", bufs=16))
        psap = ctx.enter_context(tc.tile_pool(name="psap", bufs=2, space="PSUM"))
        pssp = ctx.enter_context(tc.tile_pool(name="pssp", bufs=2, space="PSUM"))
        ctxp = ctx.enter_context(tc.tile_pool(name="ctxp", bufs=1, space="PSUM"))

        # DMA order is the cold-start critical path: Wq first, then hs chunk
        # 0, so the first projection chain can begin ~9us in; everything else
        # follows.
        # First Wq/hs transfers land in halves so the opening projection
        # chain can start accumulating d-tiles 0-7 ~5us sooner.
        wq_sb = wqp.tile([128, NDT * 256], BF, tag="wq")
        nc.sync.dma_start(out=wq_sb[:, 0:NDT * 128], in_=wqr_d[:, 0:NDT * 128])
        hst = [[None] * NSC for _ in range(B)]
        for b in range(B):
            for sc in range(NSC):
                hst[b][sc] = hstp.tile([128, NDT * 512], BF, tag="hst",
                                       name=f"hst{b}_{sc}")
        nc.sync.dma_start(out=hst[0][0][:, 0:NDT * 256],
                          in_=hsr_d[0, 0, :, 0:NDT * 256])
        nc.sync.dma_start(out=wq_sb[:, NDT * 128:], in_=wqr_d[:, NDT * 128:])
        nc.sync.dma_start(out=hst[0][0][:, NDT * 256:],
                          in_=hsr_d[0, 0, :, NDT * 256:])
        wkv_sb = wqp.tile([128, NDT * 128], BF, tag="wkv")
        nc.sync.dma_start(out=wkv_sb[:], in_=wkvr_d[:])
        ident = const.tile([128, 128], BF, tag="ident")
        nc.sync.dma_start(out=ident[:], in_=id_d[:])
        bq_sb = const.tile([128, 2], F32, tag="bq")
        nc.sync.dma_start(out=bq_sb[:], in_=bq_d[:])
        bkv_sb = const.tile([128, 1], F32, tag="bkv")
        nc.sync.dma_start(out=bkv_sb[:], in_=bkv_d[:])
        zb = const.tile([128, 1], F32, tag="zb")
        nc.vector.memset(zb[:], 0.0)

        # PE p-state warmup: the tensor engine only reaches full clock after
        # ~3us of continuous work, and the first real chain otherwise pays
        # the slow ramp right when ACT is starved for scores.  Burn the
        # initial weight/hs DMA wait (~11us) on dummy matmuls over a memset
        # scratch tile so the ramp completes before real work arrives.
        scr = const.tile([128, 512], BF, tag="scr")
        nc.vector.memset(scr[:], 0.0)
        for _ in range(24):
            ps = psap.tile([128, 512], F32, tag="ps")
            nc.tensor.matmul(ps[:], scr[:, 0:128], scr[:], start=True,
                             stop=True)
        for b in range(B):
            for sc in range(NSC):
                if (b, sc) == (0, 0):
                    continue
                nc.sync.dma_start(out=hst[b][sc][:], in_=hsr_d[b, sc])

        qT = [[None, None] for _ in range(B)]
        kvT = [None] * B
        kth = [None] * B
        # V tiles pre-created with their ones column set while the DMAs of
        # the first hs chunks are still in flight (DVE is idle then).
        v1 = [[None] * NKT for _ in range(B)]
        for b in range(B):
            for kt in range(NKT):
                v = v1p.tile([128, 65], BF, tag="v1", name=f"v1_{b}_{kt}")
                nc.vector.memset(v[:, 64:65], 1.0)
                v1[b][kt] = v

        def init_b(b):
            qT[b][0] = qtp.tile([128, S], BF, tag="qt", name=f"qT{b}_0")
            qT[b][1] = qtp.tile([128, S], BF, tag="qt", name=f"qT{b}_1")
            kvT[b] = kvp.tile([128, S], BF, tag="kv", name=f"kvT{b}")
            kth[b] = kthp.tile([128, S], BF, tag="kth", name=f"kth{b}")

        def q_chain(b, sc, qc):
            hs_t = hst[b][sc]
            c0 = sc * 512
            ps = psap.tile([128, 512], F32, tag="ps")
            for dt_ in range(NDT):
                nc.tensor.matmul(
                    ps[:],
                    wq_sb[:, dt_ * 256 + qc * 128:dt_ * 256 + (qc + 1) * 128],
                    hs_t[:, dt_ * 512:(dt_ + 1) * 512],
                    start=(dt_ == 0), stop=(dt_ == NDT - 1),
                )
                yield
            nc.vector.tensor_scalar_add(
                qT[b][qc][:, c0:c0 + 512], ps[:], bq_sb[:, qc:qc + 1])
            yield

        def chunk_q0kv(b, sc):
            """Q heads 0/1 + KV projections, kth copy and V tiles for hs
            chunk sc.  Everything attention on heads 0/1 needs; the heads
            2/3 projection (q_chain qc=1) can run much later."""
            hs_t = hst[b][sc]
            c0 = sc * 512
            yield from q_chain(b, sc, 0)
            ps = psap.tile([128, 512], F32, tag="ps")
            for dt_ in range(NDT):
                nc.tensor.matmul(
                    ps[:], wkv_sb[:, dt_ * 128:(dt_ + 1) * 128],
                    hs_t[:, dt_ * 512:(dt_ + 1) * 512],
                    start=(dt_ == 0), stop=(dt_ == NDT - 1),
                )
                yield
            nc.vector.tensor_scalar_add(
                kvT[b][:, c0:c0 + 512], ps[:], bkv_sb[:])
            yield
            # K^T rows shifted to partitions 64:127 for odd heads; issued
            # from the Pool queue so the SP queue (hsT loads) is not blocked.
            nc.gpsimd.dma_start(out=kth[b][64:128, c0:c0 + 512],
                                in_=kvT[b][0:64, c0:c0 + 512])
            yield
            # V natural tiles [s_k 128, 64]; the ones column was pre-set.
            for kt in range(sc * 4, sc * 4 + 4):
                pst = psap.tile([128, 512], BF, tag="ps", name=f"pst{b}_{kt}")
                nc.tensor.transpose(
                    pst[:, 0:64], kvT[b][64:128, kt * 128:(kt + 1) * 128],
                    ident[64:128, 64:128],
                )
                yield
                nc.vector.tensor_copy(v1[b][kt][:, 0:64], pst[:, 0:64])
                yield

        def gen_a(b):
            init_b(b)
            for sc in range(NSC):
                yield from chunk_q0kv(b, sc)
                yield from q_chain(b, sc, 1)

        # ---- phase C machinery ----
        outt = {0: [None] * (NQC * QTPC), 1: [None] * (NQC * QTPC)}
        exs = {}
        ctxs = {}

        def coff(qi):
            # qi 0..6 packed in bank 0; qi 7 at the bank-1 boundary so no
            # accumulation group straddles a PSUM bank.
            return qi * 65 if qi < 7 else 512

        def ex_stat(bkey, kt, qi):
            for ex, qi0, nqi in exs[bkey][kt]:
                if qi0 <= qi < qi0 + nqi:
                    j = qi - qi0
                    return ex[:, j * 128:(j + 1) * 128]
            raise AssertionError(f"no exp span for {bkey} kt={kt} qi={qi}")

        def pv_chunk(bkey, it, swap=False):
            # it 0..15: two passes of 8 k-tiles per q-tile qi = it//2.
            # swap=True consumes k-tiles 8..15 on the first pass (used when
            # the producing chunk emitted its exps in swapped order).
            b, qc, h = bkey
            ctx_t = ctxs[bkey]
            qi = it // 2
            base = (it % 2) * 8
            if swap:
                base = 8 - base
            for k2 in range(8):
                kt = base + k2
                nc.tensor.matmul(
                    ctx_t[:, coff(qi):coff(qi) + 65],
                    ex_stat(bkey, kt, qi),
                    v1[b][kt][:],
                    start=(it % 2 == 0 and k2 == 0),
                    stop=(it % 2 == 1 and k2 == 7),
                )

        def normalize_qi(bkey, qi):
            b, qc, h = bkey
            ctx_t = ctxs[bkey]
            qtile = qc * QTPC + qi
            if h == 0:
                outt[b][qtile] = outp.tile([128, MCOLS], F32, tag="out",
                                           name=f"outt{b}_{qtile}")
            rec = recp.tile([128, 1], F32, tag="rec")
            nc.vector.reciprocal(
                rec[:], ctx_t[:, coff(qi) + 64:coff(qi) + 65])
            nc.vector.tensor_scalar_mul(
                outt[b][qtile][:, h * 64:(h + 1) * 64],
                ctx_t[:, coff(qi):coff(qi) + 64], rec[:])
            if h == QH - 1:
                nc.sync.dma_start(
                    out=out_d[b, qtile * 128:(qtile + 1) * 128, :],
                    in_=outt[b][qtile][:])

        def begin_chunk(bkey):
            b, qc, h = bkey
            ctxs[bkey] = ctxp.tile([128, 577], F32, tag="ctx",
                                   name=f"ctx{b}_{qc}_{h}")
            exs[bkey] = []

        def score_exp(bkey, kt, qi0=0, nqi=QTPC):
            b, qc, h = bkey
            r0 = (h % 2) * 64
            kmat = kvT[b] if r0 == 0 else kth[b]
            qt = qT[b][h // 2]
            w = nqi * 128
            pss = pssp.tile([128, 1024], F32, tag="pss")
            for j in range(0, w, 512):
                jw = min(512, w - j)
                q0 = qc * 1024 + qi0 * 128 + j
                nc.tensor.matmul(
                    pss[:, j:j + jw],
                    kmat[r0:r0 + 64, kt * 128:(kt + 1) * 128],
                    qt[r0:r0 + 64, q0:q0 + jw],
                    start=True, stop=True,
                )
            ex = expp.tile([128, 1024], BF, tag="ex")
            nc.scalar.activation(ex[:, 0:w], pss[:, 0:w], Exp, bias=zb[:])
            while len(exs[bkey]) <= kt:
                exs[bkey].append([])
            exs[bkey][kt].append((ex, qi0, nqi))

        def finish_chunk(bkey):
            del exs[bkey]
            del ctxs[bkey]

        def advance(gen, n):
            if gen is None:
                return None
            for _ in range(n):
                if next(gen, StopIteration) is StopIteration:
                    return None
            return gen

        # ---- emission ----
        # Batch 0 prologue: after hs chunk 0's Q0/KV projections, scores/exp
        # for (qc0, h0) start immediately on the q/k tiles that exist,
        # widening as later chunks land.  Each 4-iteration phase interleaves
        # exactly the projection work the NEXT phase needs, so batch 0's
        # bulk is spread across the whole prologue instead of serializing
        # in front of it.
        init_b(0)
        for _ in chunk_q0kv(0, 0):
            pass
        key00 = (0, 0, 0)
        begin_chunk(key00)
        phases = [
            (range(4), 0, 4, chunk_q0kv(0, 1), 11),   # k 0:512 x q 0:512
            (range(4), 4, 4, chunk_q0kv(0, 2), 11),   # k 0:512 x q 512:1024
            (range(4, 8), 0, 8, chunk_q0kv(0, 3), 11),
            (range(8, 12), 0, 8, q_chain(0, 0, 1), 5),
            (range(12, 16), 0, 8, q_chain(0, 1, 1), 5),
        ]
        for kts, qi0, nqi, g, rate in phases:
            for kt in kts:
                score_exp(key00, kt, qi0, nqi)
                g = advance(g, rate)
            if g is not None:
                for _ in g:
                    pass

        # Steady state: remaining 7 chunks of batch 0 with the deferred
        # batch-0 head-2/3 projections and batch 1's full projection phase
        # interleaved at an adaptive rate, then batch 1's chunks.
        order0 = [(0, qc, h) for qc in range(NQC) for h in range(QH)][1:]
        order1 = [(1, qc, h) for qc in range(NQC) for h in range(QH)]

        def steady_units():
            yield from q_chain(0, 2, 1)
            yield from q_chain(0, 3, 1)
            init_b(1)
            for sc in range(NSC):
                yield from chunk_q0kv(1, sc)
                if sc < 2:
                    # heads 2/3 projections for hs chunks 2,3 are deferred
                    # into batch 1's own attention loop (needed at its
                    # iteration 96, not before it starts).
                    yield from q_chain(1, sc, 1)

        g1 = steady_units()
        prev = key00
        for bkey in order0:
            begin_chunk(bkey)
            for kt in range(NKT):
                score_exp(bkey, kt)
                pv_chunk(prev, kt)
                if kt % 2 == 1:
                    normalize_qi(prev, kt // 2)
                g1 = advance(g1, 2)
            finish_chunk(prev)
            prev = bkey
        # flush batch 1 projections before its attention begins
        if g1 is not None:
            for _ in g1:
                pass
        def b1_tail():
            yield from q_chain(1, 2, 1)
            yield from q_chain(1, 3, 1)

        gt = b1_tail()
        for bkey in order1:
            last = bkey == order1[-1]
            begin_chunk(bkey)
            for i, kt in enumerate(range(NKT)):
                # The final chunk emits k-tiles 8..15 first so its PV flush
                # (which consumes the late k-tiles on odd passes) is never
                # waiting on the exp backlog at the very end.
                score_exp(bkey, (kt + 8) % NKT if last else kt)
                pv_chunk(prev, kt)
                if kt % 2 == 1:
                    normalize_qi(prev, kt // 2)
                gt = advance(gt, 2)
            finish_chunk(prev)
            prev = bkey
        for it in range(NKT):
            pv_chunk(prev, it, swap=True)
            if it % 2 == 1:
                normalize_qi(prev, it // 2)
        finish_chunk(prev)

    nc.compile()
    return nc


def make_in_maps(hidden_states, Wq, bq, Wk, bk, Wv, bv):
    bf = mybir.dt.np(BF)
    hs = np.asarray(hidden_states, dtype=np.float32)
    Wq = np.asarray(Wq, dtype=np.float32)
    bq = np.asarray(bq, dtype=np.float32)
    Wk = np.asarray(Wk, dtype=np.float32)
    bk = np.asarray(bk, dtype=np.float32)
    Wv = np.asarray(Wv, dtype=np.float32)
    bv = np.asarray(bv, dtype=np.float32)
    sc = 1.0 / np.sqrt(np.float32(HD))
    # [b, sc, p(d in tile), t(d tile), j(s in chunk)] -> [2, 4, 128, 8192]
    hsr = np.ascontiguousarray(
        hs.reshape(B, NSC, 512, NDT, 128).transpose(0, 1, 4, 3, 2)
        .reshape(B, NSC, 128, NDT * 512).astype(bf))
    ident = np.eye(128, dtype=bf)
    in_maps = []
    for c in range(NCORES):
        qs = slice(c * MCOLS, (c + 1) * MCOLS)
        ks = slice(c * HD, (c + 1) * HD)
        wqs = (Wq[:, qs] * sc).astype(bf)
        wqr = np.ascontiguousarray(
            wqs.reshape(NDT, 128, MCOLS).transpose(1, 0, 2)
            .reshape(128, NDT * MCOLS))
        wkvs = np.concatenate([Wk[:, ks], Wv[:, ks]], axis=1).astype(bf)
        wkvr = np.ascontiguousarray(
            wkvs.reshape(NDT, 128, 128).transpose(1, 0, 2)
            .reshape(128, NDT * 128))
        bq_c = np.ascontiguousarray((bq[qs] * sc).reshape(2, 128).T)
        in_maps.append({
            "hsr": hsr,
            "wqr": wqr,
            "wkvr": wkvr,
            "bq": bq_c,
            "bkv": np.concatenate([bk[ks], bv[ks]]).reshape(128, 1),
            "ident": ident,
        })
    return in_maps


_NC_CACHE = {}


def get_nc():
    if "nc" not in _NC_CACHE:
        _NC_CACHE["nc"] = build_nc()
    return _NC_CACHE["nc"]


def kernel(hidden_states, Wq, bq, Wk, bk, Wv, bv):
    nc = get_nc()
    in_maps = make_in_maps(hidden_states, Wq, bq, Wk, bk, Wv, bv)
    res = run_bass_kernel_spmd(nc, in_maps, list(range(NCORES)))
    outs = [np.asarray(r["out"], dtype=np.float32) for r in res.results]
    return np.concatenate(outs, axis=-1)


# revision 38
# speedup vs baseline: 1.0244x; 1.0005x over previous
"""Trainium2 Bass kernel for GroupedQueryAttention (v3, bf16, early-start).

Sharding: 8 cores; core c owns KV head g=c and Q heads 4c..4c+3, both batch
elements. Each core computes its [2, 2048, 256] output slice; host concats.

Design:
  * hs is pre-transposed AND pre-cast to bf16 on the host (hsr layout:
    [b, s-chunk, d-in-tile(128), d-tile(16) x s(512)]), removing all 512
    PE transposes and their DVE drains from the device program.
  * All PE operands are bf16 (1.0 cycles/row at any ap size), PSUM fp32.
  * PV uses expS^T tiles as the STATIONARY and natural [V|1] as the moving
    ([128 k, 65]): output is ctx in natural [q, d] orientation with the
    softmax denominator in column 64 -> no output transposes, and PV costs
    65 rows per (k-tile, q-tile) instead of 512 per (k-tile, 512q).
  * exp runs on ACT in [128, 1024] chunks (double-buffered PSUM); ACT is
    the global bottleneck (~267us busy), so the schedule keeps it saturated:
    - early start: attention chunk (qc0,h0) k-tiles 0..7 begin right after
      hs-chunks 0,1 are projected; hs-chunks 2,3 interleave into those
      iterations, so the first exp fires ~35us earlier than a sequential
      phase-A/phase-C split;
    - batch 1's entire projection phase is interleaved into batch 0's
      attention loop (~2 PE instructions per k-tile iteration);
    - PV for chunk (qc,h) is software-pipelined into the scores/exp loop of
      the next chunk so ctx accumulation groups stay sequential per PSUM
      zero region (hardware allows one open group per 2KB bank).
"""

import sys
from contextlib import ExitStack

import numpy as np

sys.path.insert(0, "/opt/trn_rl_repo")

import concourse.bass as bass  # noqa: E402
import concourse.bacc as bacc  # noqa: E402
import concourse.tile as tile  # noqa: E402
from concourse import mybir  # noqa: E402
from concourse.bass_utils import run_bass_kernel_spmd  # noqa: E402

B = 2
S = 2048
D = 2048
HD = 64
NCORES = 8
QH = 4           # q heads per core
MCOLS = QH * HD  # 256 output cols per core

BF = mybir.dt.bfloat16
F32 = mybir.dt.float32
Exp = mybir.ActivationFunctionType.Exp

NDT = 16         # d tiles of 128
NSC = 4          # s chunks of 512 per batch
NKT = 16         # s_k tiles of 128
NQC = 2          # q chunks of 1024 per batch
QTPC = 8         # q tiles of 128 per q chunk


def build_nc():
    nc = bacc.Bacc("TRN2", target_bir_lowering=False, debug=False)

    hsr_d = nc.dram_tensor("hsr", [B, NSC, 128, NDT * 512], BF,
                           kind="ExternalInput")
    wqr_d = nc.dram_tensor("wqr", [128, NDT * 256], BF, kind="ExternalInput")
    wkvr_d = nc.dram_tensor("wkvr", [128, NDT * 128], BF, kind="ExternalInput")
    bq_d = nc.dram_tensor("bq", [128, 2], F32, kind="ExternalInput")
    bkv_d = nc.dram_tensor("bkv", [128, 1], F32, kind="ExternalInput")
    id_d = nc.dram_tensor("ident", [128, 128], BF, kind="ExternalInput")
    out_d = nc.dram_tensor("out", [B, S, MCOLS], F32, kind="ExternalOutput")

    with tile.TileContext(nc) as tc, ExitStack() as ctx:
        const = ctx.enter_context(tc.tile_pool(name="const", bufs=1))
        wqp = ctx.enter_context(tc.tile_pool(name="wqp", bufs=1))
        hstp = ctx.enter_context(tc.tile_pool(name="hstp", bufs=4))
        qtp = ctx.enter_context(tc.tile_pool(name="qtp", bufs=4))
        kvp = ctx.enter_context(tc.tile_pool(name="kvp", bufs=2))
        kthp = ctx.enter_context(tc.tile_pool(name="kthp", bufs=2))
        v1p = ctx.enter_context(tc.tile_pool(name="v1p", bufs=2 * NKT))
        expp = ctx.enter_context(tc.tile_pool(name="expp", bufs=38))
        recp = ctx.enter_context(tc.tile_pool(name="recp", bufs=4))
        outp = ctx.enter_context(tc.tile_pool(name="outp", bufs=16))
        psap = ctx.enter_context(tc.tile_pool(name="psap", bufs=2, space="PSUM"))
        pssp = ctx.enter_context(tc.tile_pool(name="pssp", bufs=2, space="PSUM"))
        ctxp = ctx.enter_context(tc.tile_pool(name="ctxp", bufs=1, space="PSUM"))

        # DMA order is the cold-start critical path: Wq first, then hs chunk
        # 0, so the first projection chain can begin ~9us in; everything else
        # follows.
        # First Wq/hs transfers land in halves so the opening projection
        # chain can start accumulating d-tiles 0-7 ~5us sooner.
        wq_sb = wqp.tile([128, NDT * 256], BF, tag="wq")
        nc.sync.dma_start(out=wq_sb[:, 0:NDT * 128], in_=wqr_d[:, 0:NDT * 128])
        hst = [[None] * NSC for _ in range(B)]
        for b in range(B):
            for sc in range(NSC):
                hst[b][sc] = hstp.tile([128, NDT * 512], BF, tag="hst",
                                       name=f"hst{b}_{sc}")
        nc.sync.dma_start(out=hst[0][0][:, 0:NDT * 256],
                          in_=hsr_d[0, 0, :, 0:NDT * 256])
        nc.sync.dma_start(out=wq_sb[:, NDT * 128:], in_=wqr_d[:, NDT * 128:])
        nc.sync.dma_start(out=hst[0][0][:, NDT * 256:],
                          in_=hsr_d[0, 0, :, NDT * 256:])
        wkv_sb = wqp.tile([128, NDT * 128], BF, tag="wkv")
        nc.sync.dma_start(out=wkv_sb[:], in_=wkvr_d[:])
        ident = const.tile([128, 128], BF, tag="ident")
        nc.sync.dma_start(out=ident[:], in_=id_d[:])
        bq_sb = const.tile([128, 2], F32, tag="bq")
        nc.sync.dma_start(out=bq_sb[:], in_=bq_d[:])
        bkv_sb = const.tile([128, 1], F32, tag="bkv")
        nc.sync.dma_start(out=bkv_sb[:], in_=bkv_d[:])
        zb = const.tile([128, 1], F32, tag="zb")
        nc.vector.memset(zb[:], 0.0)

        # PE p-state warmup: the tensor engine only reaches full clock after
        # ~3us of continuous work, and the first real chain otherwise pays
        # the slow ramp right when ACT is starved for scores.  Burn the
        # initial weight/hs DMA wait (~11us) on dummy matmuls over a memset
        # scratch tile so the ramp completes before real work arrives.
        scr = const.tile([128, 512], BF, tag="scr")
        nc.vector.memset(scr[:], 0.0)
        for _ in range(24):
            ps = psap.tile([128, 512], F32, tag="ps")
            nc.tensor.matmul(ps[:], scr[:, 0:128], scr[:], start=True,
                             stop=True)
        for b in range(B):
            for sc in range(NSC):
                if (b, sc) == (0, 0):
                    continue
                nc.sync.dma_start(out=hst[b][sc][:], in_=hsr_d[b, sc])

        qT = [[None, None] for _ in range(B)]
        kvT = [None] * B
        kth = [None] * B
        # V tiles pre-created with their ones column set while the DMAs of
        # the first hs chunks are still in flight (DVE is idle then).
        v1 = [[None] * NKT for _ in range(B)]
        for b in range(B):
            for kt in range(NKT):
                v = v1p.tile([128, 65], BF, tag="v1", name=f"v1_{b}_{kt}")
                nc.vector.memset(v[:, 64:65], 1.0)
                v1[b][kt] = v

        def init_b(b):
            qT[b][0] = qtp.tile([128, S], BF, tag="qt", name=f"qT{b}_0")
            qT[b][1] = qtp.tile([128, S], BF, tag="qt", name=f"qT{b}_1")
            kvT[b] = kvp.tile([128, S], BF, tag="kv", name=f"kvT{b}")
            kth[b] = kthp.tile([128, S], BF, tag="kth", name=f"kth{b}")

        def q_chain(b, sc, qc):
            hs_t = hst[b][sc]
            c0 = sc * 512
            ps = psap.tile([128, 512], F32, tag="ps")
            for dt_ in range(NDT):
                nc.tensor.matmul(
                    ps[:],
                    wq_sb[:, dt_ * 256 + qc * 128:dt_ * 256 + (qc + 1) * 128],
                    hs_t[:, dt_ * 512:(dt_ + 1) * 512],
                    start=(dt_ == 0), stop=(dt_ == NDT - 1),
                )
                yield
            nc.vector.tensor_scalar_add(
                qT[b][qc][:, c0:c0 + 512], ps[:], bq_sb[:, qc:qc + 1])
            yield

        def chunk_q0kv(b, sc):
            """Q heads 0/1 + KV projections, kth copy and V tiles for hs
            chunk sc.  Everything attention on heads 0/1 needs; the heads
            2/3 projection (q_chain qc=1) can run much later."""
            hs_t = hst[b][sc]
            c0 = sc * 512
            yield from q_chain(b, sc, 0)
            ps = psap.tile([128, 512], F32, tag="ps")
            for dt_ in range(NDT):
                nc.tensor.matmul(
                    ps[:], wkv_sb[:, dt_ * 128:(dt_ + 1) * 128],
                    hs_t[:, dt_ * 512:(dt_ + 1) * 512],
                    start=(dt_ == 0), stop=(dt_ == NDT - 1),
                )
                yield
            nc.vector.tensor_scalar_add(
                kvT[b][:, c0:c0 + 512], ps[:], bkv_sb[:])
            yield
            # K^T rows shifted to partitions 64:127 for odd heads; issued
            # from the Pool queue so the SP queue (hsT loads) is not blocked.
            nc.gpsimd.dma_start(out=kth[b][64:128, c0:c0 + 512],
                                in_=kvT[b][0:64, c0:c0 + 512])
            yield
            # V natural tiles [s_k 128, 64]; the ones column was pre-set.
            for kt in range(sc * 4, sc * 4 + 4):
                pst = psap.tile([128, 512], BF, tag="ps", name=f"pst{b}_{kt}")
                nc.tensor.transpose(
                    pst[:, 0:64], kvT[b][64:128, kt * 128:(kt + 1) * 128],
                    ident[64:128, 64:128],
                )
                yield
                nc.vector.tensor_copy(v1[b][kt][:, 0:64], pst[:, 0:64])
                yield

        def gen_a(b):
            init_b(b)
            for sc in range(NSC):
                yield from chunk_q0kv(b, sc)
                yield from q_chain(b, sc, 1)

        # ---- phase C machinery ----
        outt = {0: [None] * (NQC * QTPC), 1: [None] * (NQC * QTPC)}
        exs = {}
        ctxs = {}

        def coff(qi):
            # qi 0..6 packed in bank 0; qi 7 at the bank-1 boundary so no
            # accumulation group straddles a PSUM bank.
            return qi * 65 if qi < 7 else 512

        def ex_stat(bkey, kt, qi):
            for ex, qi0, nqi in exs[bkey][kt]:
                if qi0 <= qi < qi0 + nqi:
                    j = qi - qi0
                    return ex[:, j * 128:(j + 1) * 128]
            raise AssertionError(f"no exp span for {bkey} kt={kt} qi={qi}")

        def pv_chunk(bkey, it, swap=False):
            # it 0..15: two passes of 8 k-tiles per q-tile qi = it//2.
            # swap=True consumes k-tiles 8..15 on the first pass (used when
            # the producing chunk emitted its exps in swapped order).
            b, qc, h = bkey
            ctx_t = ctxs[bkey]
            qi = it // 2
            base = (it % 2) * 8
            if swap:
                base = 8 - base
            for k2 in range(8):
                kt = base + k2
                nc.tensor.matmul(
                    ctx_t[:, coff(qi):coff(qi) + 65],
                    ex_stat(bkey, kt, qi),
                    v1[b][kt][:],
                    start=(it % 2 == 0 and k2 == 0),
                    stop=(it % 2 == 1 and k2 == 7),
                )

        def normalize_qi(bkey, qi):
            b, qc, h = bkey
            ctx_t = ctxs[bkey]
            qtile = qc * QTPC + qi
            if h == 0:
                outt[b][qtile] = outp.tile([128, MCOLS], F32, tag="out",
                                           name=f"outt{b}_{qtile}")
            rec = recp.tile([128, 1], F32, tag="rec")
            nc.vector.reciprocal(
                rec[:], ctx_t[:, coff(qi) + 64:coff(qi) + 65])
            nc.vector.tensor_scalar_mul(
                outt[b][qtile][:, h * 64:(h + 1) * 64],
                ctx_t[:, coff(qi):coff(qi) + 64], rec[:])
            if h == QH - 1:
                nc.sync.dma_start(
                    out=out_d[b, qtile * 128:(qtile + 1) * 128, :],
                    in_=outt[b][qtile][:])

        def begin_chunk(bkey):
            b, qc, h = bkey
            ctxs[bkey] = ctxp.tile([128, 577], F32, tag="ctx",
                                   name=f"ctx{b}_{qc}_{h}")
            exs[bkey] = []

        def score_exp(bkey, kt, qi0=0, nqi=QTPC):
            b, qc, h = bkey
            r0 = (h % 2) * 64
            kmat = kvT[b] if r0 == 0 else kth[b]
            qt = qT[b][h // 2]
            w = nqi * 128
            pss = pssp.tile([128, 1024], F32, tag="pss")
            for j in range(0, w, 512):
                jw = min(512, w - j)
                q0 = qc * 1024 + qi0 * 128 + j
                nc.tensor.matmul(
                    pss[:, j:j + jw],
                    kmat[r0:r0 + 64, kt * 128:(kt + 1) * 128],
                    qt[r0:r0 + 64, q0:q0 + jw],
                    start=True, stop=True,
                )
            ex = expp.tile([128, 1024], BF, tag="ex")
            nc.scalar.activation(ex[:, 0:w], pss[:, 0:w], Exp, bias=zb[:])
            while len(exs[bkey]) <= kt:
                exs[bkey].append([])
            exs[bkey][kt].append((ex, qi0, nqi))

        def finish_chunk(bkey):
            del exs[bkey]
            del ctxs[bkey]

        def advance(gen, n):
            if gen is None:
                return None
            for _ in range(n):
                if next(gen, StopIteration) is StopIteration:
                    return None
            return gen

        # ---- emission ----
        # Batch 0 prologue: after hs chunk 0's Q0/KV projections, scores/exp
        # for (qc0, h0) start immediately on the q/k tiles that exist,
        # widening as later chunks land.  Each 4-iteration phase interleaves
        # exactly the projection work the NEXT phase needs, so batch 0's
        # bulk is spread across the whole prologue instead of serializing
        # in front of it.
        init_b(0)
        for _ in chunk_q0kv(0, 0):
            pass
        key00 = (0, 0, 0)
        begin_chunk(key00)
        phases = [
            (range(4), 0, 4, chunk_q0kv(0, 1), 11),   # k 0:512 x q 0:512
            (range(4), 4, 4, chunk_q0kv(0, 2), 11),   # k 0:512 x q 512:1024
            (range(4, 8), 0, 8, chunk_q0kv(0, 3), 11),
            (range(8, 12), 0, 8, q_chain(0, 0, 1), 5),
            (range(12, 16), 0, 8, q_chain(0, 1, 1), 5),
        ]
        for kts, qi0, nqi, g, rate in phases:
            for kt in kts:
                score_exp(key00, kt, qi0, nqi)
                g = advance(g, rate)
            if g is not None:
                for _ in g:
                    pass

        # Steady state: remaining 7 chunks of batch 0 with the deferred
        # batch-0 head-2/3 projections and batch 1's full projection phase
        # interleaved at an adaptive rate, then batch 1's chunks.
        order0 = [(0, qc, h) for qc in range(NQC) for h in range(QH)][1:]
        order1 = [(1, qc, h) for qc in range(NQC) for h in range(QH)]

        def steady_units():
            yield from q_chain(0, 2, 1)
            yield from q_chain(0, 3, 1)
            init_b(1)
            for sc in range(NSC):
                yield from chunk_q0kv(1, sc)
                if sc < 2:
                    # heads 2/3 projections for hs chunks 2,3 are deferred
                    # into batch 1's own attention loop (needed at its
                    # iteration 96, not before it starts).
                    yield from q_chain(1, sc, 1)

        g1 = steady_units()
        prev = key00
        for bkey in order0:
            begin_chunk(bkey)
            for kt in range(NKT):
                score_exp(bkey, kt)
                pv_chunk(prev, kt)
                if kt % 2 == 1:
                    normalize_qi(prev, kt // 2)
                # 2 units/iter makes PE slightly outweigh ACT (1070 vs
                # 1038ns); dropping to 1 every 8th iter balances the two
                # (avg 1.875/iter still covers all 206 deferred units).
                g1 = advance(g1, 1 if kt % 8 == 7 else 2)
            finish_chunk(prev)
            prev = bkey
        # flush batch 1 projections before its attention begins
        if g1 is not None:
            for _ in g1:
                pass
        def b1_tail():
            yield from q_chain(1, 2, 1)
            yield from q_chain(1, 3, 1)

        gt = b1_tail()
        for bkey in order1:
            last = bkey == order1[-1]
            begin_chunk(bkey)
            for i, kt in enumerate(range(NKT)):
                # The final chunk emits k-tiles 8..15 first so its PV flush
                # (which consumes the late k-tiles on odd passes) is never
                # waiting on the exp backlog at the very end.
                score_exp(bkey, (kt + 8) % NKT if last else kt)
                pv_chunk(prev, kt)
                if kt % 2 == 1:
                    normalize_qi(prev, kt // 2)
                gt = advance(gt, 2)
            finish_chunk(prev)
            prev = bkey
        for it in range(NKT):
            pv_chunk(prev, it, swap=True)
            if it % 2 == 1:
                normalize_qi(prev, it // 2)
        finish_chunk(prev)

    nc.compile()
    return nc


def make_in_maps(hidden_states, Wq, bq, Wk, bk, Wv, bv):
    bf = mybir.dt.np(BF)
    hs = np.asarray(hidden_states, dtype=np.float32)
    Wq = np.asarray(Wq, dtype=np.float32)
    bq = np.asarray(bq, dtype=np.float32)
    Wk = np.asarray(Wk, dtype=np.float32)
    bk = np.asarray(bk, dtype=np.float32)
    Wv = np.asarray(Wv, dtype=np.float32)
    bv = np.asarray(bv, dtype=np.float32)
    sc = 1.0 / np.sqrt(np.float32(HD))
    # [b, sc, p(d in tile), t(d tile), j(s in chunk)] -> [2, 4, 128, 8192]
    hsr = np.ascontiguousarray(
        hs.reshape(B, NSC, 512, NDT, 128).transpose(0, 1, 4, 3, 2)
        .reshape(B, NSC, 128, NDT * 512).astype(bf))
    ident = np.eye(128, dtype=bf)
    in_maps = []
    for c in range(NCORES):
        qs = slice(c * MCOLS, (c + 1) * MCOLS)
        ks = slice(c * HD, (c + 1) * HD)
        wqs = (Wq[:, qs] * sc).astype(bf)
        wqr = np.ascontiguousarray(
            wqs.reshape(NDT, 128, MCOLS).transpose(1, 0, 2)
            .reshape(128, NDT * MCOLS))
        wkvs = np.concatenate([Wk[:, ks], Wv[:, ks]], axis=1).astype(bf)
        wkvr = np.ascontiguousarray(
            wkvs.reshape(NDT, 128, 128).transpose(1, 0, 2)
            .reshape(128, NDT * 128))
        bq_c = np.ascontiguousarray((bq[qs] * sc).reshape(2, 128).T)
        in_maps.append({
            "hsr": hsr,
            "wqr": wqr,
            "wkvr": wkvr,
            "bq": bq_c,
            "bkv": np.concatenate([bk[ks], bv[ks]]).reshape(128, 1),
            "ident": ident,
        })
    return in_maps


_NC_CACHE = {}


def get_nc():
    if "nc" not in _NC_CACHE:
        _NC_CACHE["nc"] = build_nc()
    return _NC_CACHE["nc"]


def kernel(hidden_states, Wq, bq, Wk, bk, Wv, bv):
    nc = get_nc()
    in_maps = make_in_maps(hidden_states, Wq, bq, Wk, bk, Wv, bv)
    res = run_bass_kernel_spmd(nc, in_maps, list(range(NCORES)))
    outs = [np.asarray(r["out"], dtype=np.float32) for r in res.results]
    return np.concatenate(outs, axis=-1)
